# revision 1
# baseline (speedup 1.0000x reference)
"""Trainium2 Bass kernel for a continuous-time diagonal SSM layer (S5/S4D-style).

Math (see reference):
    A = exp(Lambda * step)                 (P,) complex, |A| = r, arg = theta
    Bu[t] = B_bar @ u[t]                   (L, P) complex
    x[t]  = A * x[t-1] + Bu[t]             diagonal complex scan
    ys    = 2 Re(C_tilde @ x) + D * u

Kernel strategy (8 cores, sequence-parallel over L):
  * core i owns a contiguous span of LC = L/8 timesteps, chunked by T=512.
  * rotation trick: with A = r e^{i.th}, define y[t] = e^{-i.th.t} x[t].
    Then y obeys a REAL per-partition recurrence y[t] = r y[t-1] + w[t],
    w[t] = e^{-i.th.t} Bu[t], which maps onto the DVE tensor_tensor_scan
    instruction (state = data0*state + data1 along the free dim).
  * layout: state-major [128 modes (partitions) x time (free)].  Bu^T comes
    from PE matmuls vs host-pretransposed u^T; output matmul produces
    time-major ys directly (lhsT = x tiles), D*u is folded in as a
    diagonal matmul from u^T.  Phase tables (cos/sin of th*s, s in [0,T))
    are chunk-local, host-precomputed in f64; chunk carries chain through
    the scan `initial` operand after a tiny per-chunk basis rotation.
  * cross-core carry: each core AllGathers its span-final scan state (1KB),
    combines them with host-precomputed decay weights, and adds the
    correction r^{s+1}*G into y before the output projection.
  * dtypes: bf16 elementwise domain, f32 scan multiplier (bf16 cannot
    represent slow decay rates like r=0.99995), float32r matmuls.
"""

import numpy as np
import ml_dtypes

import concourse.bass as bass
import concourse.bacc as bacc
import concourse.tile as tile
import concourse.mybir as mybir
from concourse.bass_utils import run_bass_kernel_spmd

F32 = mybir.dt.float32
F32R = mybir.dt.float32r
BF16 = mybir.dt.bfloat16
NPBF16 = ml_dtypes.bfloat16
AX = mybir.AxisListType.X
MUL = mybir.AluOpType.mult
ADD = mybir.AluOpType.add

L, H, P = 32768, 256, 128
NCORES = 8
LC = L // NCORES          # 4096 timesteps per core
T = 512                   # chunk length
NCH = LC // T             # 8 chunks per core


def _build_kernel(single=False):
    nc = bacc.Bacc(
        "TRN2", target_bir_lowering=False, debug=False,
        enable_asserts=True, num_devices=1 if single else NCORES,
    )
    # ---------------- per-core external I/O ----------------
    ut_d = nc.dram_tensor("ut", [2, P, LC], F32R, kind="ExternalInput").ap()
    cs_d = nc.dram_tensor("cs", [P, T], BF16, kind="ExternalInput").ap()
    sn_d = nc.dram_tensor("sn", [P, T], BF16, kind="ExternalInput").ap()
    rrep_d = nc.dram_tensor("rrep", [P, T], F32, kind="ExternalInput").ap()
    rpow_d = nc.dram_tensor("rpow", [P, T], BF16, kind="ExternalInput").ap()
    btr_d = nc.dram_tensor("btr", [2, P, P], F32R, kind="ExternalInput").ap()
    bti_d = nc.dram_tensor("bti", [2, P, P], F32R, kind="ExternalInput").ap()
    crt_d = nc.dram_tensor("crt", [P, H], BF16, kind="ExternalInput").ap()
    cit_d = nc.dram_tensor("cit", [P, H], BF16, kind="ExternalInput").ap()
    dd_d = nc.dram_tensor("dd", [2, P, H], F32R, kind="ExternalInput").ap()
    mc_d = nc.dram_tensor("mc", [P, 2 * NCH], F32, kind="ExternalInput").ap()
    ctc_d = nc.dram_tensor("ctc", [P, 2], F32, kind="ExternalInput").ap()
    wgr_d = nc.dram_tensor("wgr", [P, NCORES], F32, kind="ExternalInput").ap()
    wgi_d = nc.dram_tensor("wgi", [P, NCORES], F32, kind="ExternalInput").ap()
    out_d = nc.dram_tensor("out", [LC, H], F32, kind="ExternalOutput").ap()

    with tile.TileContext(nc) as tc:
        _body(tc, nc, ut_d, cs_d, sn_d, rrep_d, rpow_d, btr_d, bti_d,
              crt_d, cit_d, dd_d, mc_d, ctc_d, wgr_d, wgi_d, out_d,
              single=single)
    nc.compile()
    return nc


def _body(tc, nc, ut_d, cs_d, sn_d, rrep_d, rpow_d, btr_d, bti_d,
          crt_d, cit_d, dd_d, mc_d, ctc_d, wgr_d, wgi_d, out_d, single=False):
    with (
        tc.tile_pool(name="const", bufs=1) as cpool,
        tc.tile_pool(name="span", bufs=1) as spool,
        tc.tile_pool(name="work", bufs=3) as wpool,
        tc.tile_pool(name="psum", bufs=2, space="PSUM") as ppool,
        tc.tile_pool(name="opsum", bufs=2, space="PSUM") as opool,
        tc.tile_pool(name="dram", bufs=1, space="DRAM") as dpool,
    ):
        # ---- constants ----
        cs_sb = cpool.tile([P, T], BF16)
        sn_sb = cpool.tile([P, T], BF16)
        rrep_sb = cpool.tile([P, T], F32)
        rpow_sb = cpool.tile([P, T], BF16)
        btr_sb = cpool.tile([P, 2, P], F32R)
        bti_sb = cpool.tile([P, 2, P], F32R)
        crt_sb = cpool.tile([P, H], BF16)
        cit_sb = cpool.tile([P, H], BF16)
        dd_sb = cpool.tile([P, 2, H], F32R)
        mc_sb = cpool.tile([P, 2 * NCH], F32)
        ctc_sb = cpool.tile([P, 2], F32)
        wgr_sb = cpool.tile([P, NCORES], F32)
        wgi_sb = cpool.tile([P, NCORES], F32)
        for dst, src in ((cs_sb, cs_d), (sn_sb, sn_d), (rrep_sb, rrep_d),
                         (rpow_sb, rpow_d), (crt_sb, crt_d), (cit_sb, cit_d),
                         (mc_sb, mc_d), (ctc_sb, ctc_d),
                         (wgr_sb, wgr_d), (wgi_sb, wgi_d)):
            nc.sync.dma_start(dst[:], src)
        for dst, src in ((btr_sb, btr_d), (bti_sb, bti_d), (dd_sb, dd_d)):
            for a in range(2):
                nc.sync.dma_start(dst[:, a, :], src[a])

        # ---- span-persistent state ----
        ut_sb = spool.tile([P, 2, LC], F32R)      # u^T, kept for D*u
        yr_sb = spool.tile([P, LC], BF16)         # scan outputs (rotated basis)
        yi_sb = spool.tile([P, LC], BF16)
        gcols = spool.tile([P, 2 * (NCH + 1)], F32)   # chunk-carry columns

        # =============== phase 1: Bu, rotate, scan ===============
        for c in range(NCH):
            t0 = c * T
            for hh in range(2):
                nc.sync.dma_start(ut_sb[:, hh, t0:t0 + T], ut_d[hh, :, t0:t0 + T])
            pbr = ppool.tile([P, T], F32, tag="pbur")
            pbi = ppool.tile([P, T], F32, tag="pbui")
            for a in range(2):
                nc.tensor.matmul(pbr[:], btr_sb[:, a, :], ut_sb[:, a, t0:t0 + T],
                                 start=(a == 0), stop=(a == 1))
            for a in range(2):
                nc.tensor.matmul(pbi[:], bti_sb[:, a, :], ut_sb[:, a, t0:t0 + T],
                                 start=(a == 0), stop=(a == 1))
            bur = wpool.tile([P, T], BF16, tag="bur")
            bui = wpool.tile([P, T], BF16, tag="bui")
            nc.scalar.copy(bur[:], pbr[:])
            nc.scalar.copy(bui[:], pbi[:])
            # w = e^{-i th s} * Bu
            m1 = wpool.tile([P, T], BF16, tag="m1")
            m2 = wpool.tile([P, T], BF16, tag="m2")
            m3 = wpool.tile([P, T], BF16, tag="m3")
            m4 = wpool.tile([P, T], BF16, tag="m4")
            wr = wpool.tile([P, T], BF16, tag="wr")
            wi = wpool.tile([P, T], BF16, tag="wi")
            nc.vector.tensor_mul(m1[:], cs_sb[:], bur[:])
            nc.gpsimd.tensor_mul(m2[:], sn_sb[:], bui[:])
            nc.vector.tensor_add(wr[:], m1[:], m2[:])
            nc.vector.tensor_mul(m3[:], cs_sb[:], bui[:])
            nc.gpsimd.tensor_mul(m4[:], sn_sb[:], bur[:])
            nc.vector.tensor_sub(wi[:], m3[:], m4[:])
            # scans
            init_r = 0.0 if c == 0 else gcols[:, 2 * c:2 * c + 1]
            init_i = 0.0 if c == 0 else gcols[:, 2 * c + 1:2 * c + 2]
            nc.vector.tensor_tensor_scan(
                yr_sb[:, t0:t0 + T], rrep_sb[:], wr[:], init_r, op0=MUL, op1=ADD)
            nc.vector.tensor_tensor_scan(
                yi_sb[:, t0:t0 + T], rrep_sb[:], wi[:], init_i, op0=MUL, op1=ADD)
            # chunk carry: g_{c+1} = e^{i th T} * y[:, last]
            if c < NCH - 1:
                ylr = yr_sb[:, t0 + T - 1:t0 + T]
                yli = yi_sb[:, t0 + T - 1:t0 + T]
                tc1 = wpool.tile([P, 1], F32, tag="tc1")
                tc2 = wpool.tile([P, 1], F32, tag="tc2")
                nc.vector.tensor_scalar_mul(tc1[:], yli[:], ctc_sb[:, 1:2])
                nc.vector.scalar_tensor_tensor(
                    gcols[:, 2 * c + 2:2 * c + 3], ylr, ctc_sb[:, 0:1], tc1[:],
                    op0=MUL, op1=mybir.AluOpType.subtract)
                nc.vector.tensor_scalar_mul(tc2[:], ylr[:], ctc_sb[:, 1:2])
                nc.vector.scalar_tensor_tensor(
                    gcols[:, 2 * c + 3:2 * c + 4], yli, ctc_sb[:, 0:1], tc2[:],
                    op0=MUL, op1=ADD)

        # =============== cross-core carry exchange ===============
        e_loc = dpool.tile([P, 2], F32)
        e_all = dpool.tile([NCORES * P, 2], F32)
        e_sb = cpool.tile([P, 2], F32)
        nc.vector.tensor_copy(e_sb[:, 0:1], yr_sb[:, LC - 1:LC])
        nc.vector.tensor_copy(e_sb[:, 1:2], yi_sb[:, LC - 1:LC])
        nc.gpsimd.dma_start(e_loc[:], e_sb[:])
        if single:
            nc.gpsimd.dma_start(e_all[0:P, :], e_loc[:])
        else:
            nc.gpsimd.collective_compute(
                "AllGather", mybir.AluOpType.bypass,
                replica_groups=[list(range(NCORES))],
                ins=[e_loc.opt()], outs=[e_all.opt()])
        eall_sb = cpool.tile([P, 2 * NCORES], F32)
        for j in range(NCORES):
            nc.gpsimd.dma_start(eall_sb[:, 2 * j:2 * j + 2], e_all[j * P:(j + 1) * P, :])
        # Cin = sum_j W'_j * E_j   (complex, W' host-precomputed per core)
        er_v = eall_sb[:, 0:2 * NCORES:2]
        ei_v = eall_sb[:, 1:2 * NCORES:2]
        ta = cpool.tile([P, NCORES], F32)
        tb = cpool.tile([P, NCORES], F32)
        cin = cpool.tile([P, 2], F32)
        nc.vector.tensor_mul(ta[:], wgr_sb[:], er_v)
        nc.vector.tensor_mul(tb[:], wgi_sb[:], ei_v)
        nc.vector.tensor_sub(ta[:], ta[:], tb[:])
        nc.vector.tensor_reduce(cin[:, 0:1], ta[:], axis=AX, op=ADD)
        nc.vector.tensor_mul(ta[:], wgi_sb[:], er_v)
        nc.vector.tensor_mul(tb[:], wgr_sb[:], ei_v)
        nc.vector.tensor_add(ta[:], ta[:], tb[:])
        nc.vector.tensor_reduce(cin[:, 1:2], ta[:], axis=AX, op=ADD)
        # G_all[:, c] = (e^{i th} A^{T c}) * Cin  for each chunk c
        gr_all = cpool.tile([P, NCH], F32)
        gi_all = cpool.tile([P, NCH], F32)
        tg = cpool.tile([P, NCH], F32)
        nc.vector.tensor_scalar_mul(gr_all[:], mc_sb[:, 0:NCH], cin[:, 0:1])
        nc.vector.tensor_scalar_mul(tg[:], mc_sb[:, NCH:2 * NCH], cin[:, 1:2])
        nc.vector.tensor_sub(gr_all[:], gr_all[:], tg[:])
        nc.vector.tensor_scalar_mul(gi_all[:], mc_sb[:, NCH:2 * NCH], cin[:, 0:1])
        nc.vector.tensor_scalar_mul(tg[:], mc_sb[:, 0:NCH], cin[:, 1:2])
        nc.vector.tensor_add(gi_all[:], gi_all[:], tg[:])

        # =============== phase 2: correct, unrotate, project ===============
        for c in range(NCH):
            t0 = c * T
            # y += r^{s+1} * G_c
            cr1 = wpool.tile([P, T], BF16, tag="cr1")
            cr2 = wpool.tile([P, T], BF16, tag="cr2")
            yrc = wpool.tile([P, T], BF16, tag="yrc")
            yic = wpool.tile([P, T], BF16, tag="yic")
            nc.scalar.mul(cr1[:], rpow_sb[:], gr_all[:, c:c + 1])
            nc.scalar.mul(cr2[:], rpow_sb[:], gi_all[:, c:c + 1])
            nc.vector.tensor_add(yrc[:], cr1[:], yr_sb[:, t0:t0 + T])
            nc.vector.tensor_add(yic[:], cr2[:], yi_sb[:, t0:t0 + T])
            # x = e^{+i th s} * y
            n1 = wpool.tile([P, T], BF16, tag="n1")
            n2 = wpool.tile([P, T], BF16, tag="n2")
            xr = wpool.tile([P, T], BF16, tag="xr")
            xi = wpool.tile([P, T], BF16, tag="xi")
            nc.vector.tensor_mul(n1[:], cs_sb[:], yrc[:])
            nc.gpsimd.tensor_mul(n2[:], sn_sb[:], yic[:])
            nc.vector.tensor_sub(xr[:], n1[:], n2[:])
            nc.gpsimd.tensor_mul(n1[:], cs_sb[:], yic[:])
            nc.vector.tensor_mul(n2[:], sn_sb[:], yrc[:])
            nc.vector.tensor_add(xi[:], n1[:], n2[:])
            # ys^T-free output: ys[t,h] = 2Re(C x)[t,h] + (D u)[t,h]
            po = opool.tile([P, 4, H], F32, tag="po")
            ob = wpool.tile([P, 4, H], F32, tag="ob")
            for a in range(4):
                sl = slice(a * P, (a + 1) * P)
                nc.tensor.matmul(po[:, a, :], xr[:, sl], crt_sb[:], start=True, stop=False)
                nc.tensor.matmul(po[:, a, :], xi[:, sl], cit_sb[:], start=False, stop=False)
                nc.tensor.matmul(po[:, a, :], ut_sb[:, 0, t0 + a * P:t0 + (a + 1) * P],
                                 dd_sb[:, 0, :], start=False, stop=False)
                nc.tensor.matmul(po[:, a, :], ut_sb[:, 1, t0 + a * P:t0 + (a + 1) * P],
                                 dd_sb[:, 1, :], start=False, stop=True)
                nc.scalar.copy(ob[:, a, :], po[:, a, :])
                nc.sync.dma_start(out_d[t0 + a * P:t0 + (a + 1) * P, :], ob[:, a, :])


_NC_CACHE = {}


def _get_nc():
    if "nc" not in _NC_CACHE:
        _NC_CACHE["nc"] = _build_kernel()
    return _NC_CACHE["nc"]


def _host_prep(Lambda_re, Lambda_im, B, C, D, log_step, input_sequence):
    """f64 host-side parameter/table preparation -> per-core input maps."""
    Lam = Lambda_re.astype(np.float64) + 1j * Lambda_im.astype(np.float64)
    step = np.exp(log_step.astype(np.float64))
    A = np.exp(Lam * step)                        # (P,)
    r = np.abs(A)
    th = np.imag(Lam * step)
    Bt = B[..., 0].astype(np.float64) + 1j * B[..., 1].astype(np.float64)
    Bbar = ((A - 1.0) / Lam)[:, None] * Bt        # (P, H)
    Ct = C[..., 0].astype(np.float64) + 1j * C[..., 1].astype(np.float64)  # (H, P)

    s = np.arange(T, dtype=np.float64)
    cs = np.cos(th[:, None] * s[None, :])
    sn = np.sin(th[:, None] * s[None, :])
    rrep = np.broadcast_to(r[:, None], (P, T))
    rpow = r[:, None] ** (s[None, :] + 1.0)

    Br, Bi = np.real(Bbar), np.imag(Bbar)
    btr = np.stack([Br.T[a * P:(a + 1) * P] for a in range(2)])   # (2, 128h, 128p)
    bti = np.stack([Bi.T[a * P:(a + 1) * P] for a in range(2)])
    crt = 2.0 * np.real(Ct).T                                     # (P, H)
    cit = -2.0 * np.imag(Ct).T
    dd = np.zeros((2, P, H), np.float64)
    for a in range(2):
        for hh in range(P):
            dd[a, hh, a * P + hh] = D[a * P + hh]
    # per-chunk carry application matrices  e^{i th} A^{T c}
    mc = np.zeros((P, 2 * NCH), np.float64)
    eA = np.exp(1j * th)
    for c in range(NCH):
        m = eA * (A ** (T * c))
        mc[:, c] = np.real(m)
        mc[:, NCH + c] = np.imag(m)
    ctc = np.stack([np.cos(th * T), np.sin(th * T)], axis=1)      # (P, 2)

    u = np.ascontiguousarray(input_sequence.astype(np.float32))
    ALC = A ** LC
    eE = np.exp(1j * th * (T - 1))     # local y -> local span-final state phase

    in_maps = []
    for i in range(NCORES):
        ui = u[i * LC:(i + 1) * LC]                               # (LC, H)
        utc = np.ascontiguousarray(ui.T).reshape(2, P, LC)
        wgr = np.zeros((P, NCORES), np.float64)
        wgi = np.zeros((P, NCORES), np.float64)
        for j in range(i):
            w = (ALC ** (i - 1 - j)) * eE
            wgr[:, j] = np.real(w)
            wgi[:, j] = np.imag(w)
        in_maps.append({
            "ut": utc.astype(np.float32),
            "cs": cs.astype(NPBF16),
            "sn": sn.astype(NPBF16),
            "rrep": np.ascontiguousarray(rrep).astype(np.float32),
            "rpow": rpow.astype(NPBF16),
            "btr": btr.astype(np.float32),
            "bti": bti.astype(np.float32),
            "crt": crt.astype(NPBF16),
            "cit": cit.astype(NPBF16),
            "dd": dd.astype(np.float32),
            "mc": mc.astype(np.float32),
            "ctc": ctc.astype(np.float32),
            "wgr": wgr.astype(np.float32),
            "wgi": wgi.astype(np.float32),
        })
    return in_maps


def kernel(Lambda_re, Lambda_im, B, C, D, log_step, input_sequence):
    in_maps = _host_prep(Lambda_re, Lambda_im, B, C, D, log_step,
                         input_sequence)
    nc = _get_nc()
    res = run_bass_kernel_spmd(nc, in_maps, list(range(NCORES)))
    out = np.concatenate([res.results[i]["out"] for i in range(NCORES)], axis=0)
    return out.astype(np.float32)


if __name__ == "__main__":
    rng = np.random.default_rng(0)
    pass



# revision 13
# speedup vs baseline: 1.0214x; 1.0214x over previous
"""Trainium2 Bass kernel for a continuous-time diagonal SSM layer (S5/S4D-style).

Math (see reference):
    A = exp(Lambda * step)                 (P,) complex, |A| = r, arg = theta
    Bu[t] = B_bar @ u[t]                   (L, P) complex
    x[t]  = A * x[t-1] + Bu[t]             diagonal complex scan
    ys    = 2 Re(C_tilde @ x) + D * u

Kernel strategy (8 cores, sequence-parallel over L):
  * core i owns a contiguous span of LC = L/8 timesteps, chunked by T=512.
  * rotation trick with a SPAN-GLOBAL basis: with A = r e^{i.th}, define
    y[t] = e^{-i.th.t} x[t] (t = span-local index).  y obeys a REAL
    per-partition recurrence y[t] = r y[t-1] + w[t], w[t] = e^{-i.th.t} Bu[t],
    mapping onto the DVE tensor_tensor_scan.  Chunk scans chain by passing
    initial = previous chunk's last output column.
  * cross-core carry: each core AllGathers its span-final scan state (1KB),
    combines with host-precomputed decay weights into a per-partition complex
    scalar Cin = cr + i*ci.  The correction x[t] += A^{t+1} Cin is applied IN
    OUTPUT SPACE: ys_corr[t,h] = sum_p TR[p,t] W1[p,h] + TI[p,t] W2[p,h]
    with TR/TI = Re/Im A^{t+1} (host tables) and W1 = crt*cr + cit*ci,
    W2 = cit*cr - crt*ci folded once after the collective - two extra PE
    matmuls per output subtile, zero extra DVE passes.
  * D*u is added on the HOST after gather (saves 8 PE matmuls per chunk).
  * engine balance (cost-model driven): DVE 297ns/tensor-tensor (bf16 2x),
    594ns/scan; Pool 1111ns/op (no PSUM access allowed!); Act copies
    754-1506ns.  Per chunk: DVE gets 3 mults + 1 add + 2 scans (p1), 2 mults
    + 2 add/sub (p2); Pool gets 2 ops per phase; Act does PSUM->SBUF copies;
    PE all matmuls; const/in/out DMAs ride SP + Act HWDGE queues (issuing
    big DMAs from gpsimd stalls the Pool ENGINE for the whole transfer).
"""

import numpy as np
import ml_dtypes

import concourse.bass as bass
import concourse.bacc as bacc
import concourse.tile as tile
import concourse.mybir as mybir
from concourse.bass_utils import run_bass_kernel_spmd

F32 = mybir.dt.float32
BF16 = mybir.dt.bfloat16
NPBF16 = ml_dtypes.bfloat16
AX = mybir.AxisListType.X
MUL = mybir.AluOpType.mult
ADD = mybir.AluOpType.add

L, H, P = 32768, 256, 128
NCORES = 8
LC = L // NCORES          # 4096 timesteps per core
T = 512                   # chunk length
NCH = LC // T             # 8 chunks per core

# weight-blob layout (free-dim offsets, all bf16, partition=128)
_WB_BTR0 = 0          # Re(Bbar).T rows   0:128  -> [128h, 128p]
_WB_BTR1 = 128        # Re(Bbar).T rows 128:256
_WB_BTI0 = 256        # Im(Bbar).T rows   0:128
_WB_BTI1 = 384        # Im(Bbar).T rows 128:256
_WB_CRT = 512         # 2*Re(C).T   [128p, 256h]
_WB_CIT = 768         # -2*Im(C).T  [128p, 256h]
_WB_LEN = 1024


def _build_kernel(single=False):
    nc = bacc.Bacc(
        "TRN2", target_bir_lowering=False, debug=False,
        enable_asserts=False, num_devices=1 if single else NCORES,
    )
    # ---------------- per-core external I/O ----------------
    ut_d = nc.dram_tensor("ut", [P, 2, LC], BF16, kind="ExternalInput").ap()
    csn_d = nc.dram_tensor("csn", [P, NCH, 2, T], BF16, kind="ExternalInput").ap()
    tt_d = nc.dram_tensor("tt", [P, NCH, 2, T], BF16, kind="ExternalInput").ap()
    wb_d = nc.dram_tensor("wb", [P, _WB_LEN], BF16, kind="ExternalInput").ap()
    fb_d = nc.dram_tensor("fb", [P, T + 2 * NCORES], F32, kind="ExternalInput").ap()
    # natural (p, a, h) order to match the SBUF tile; host re-lays out
    out_d = nc.dram_tensor("out", [NCH, P, 4, H], BF16, kind="ExternalOutput").ap()

    with tile.TileContext(nc) as tc:
        _body(tc, nc, ut_d, csn_d, tt_d, wb_d, fb_d, out_d, single=single)
    nc.compile()
    return nc


def _body(tc, nc, ut_d, csn_d, tt_d, wb_d, fb_d, out_d, single=False):
    with (
        tc.tile_pool(name="const", bufs=1) as cpool,
        tc.tile_pool(name="span", bufs=1) as spool,
        tc.tile_pool(name="work", bufs=3) as wpool,
        tc.tile_pool(name="pbu", bufs=2, space="PSUM") as ppool,
        tc.tile_pool(name="pout", bufs=2, space="PSUM") as opool,
        tc.tile_pool(name="dram", bufs=1, space="DRAM") as dpool,
    ):
        # ---- constants (SP + Act HWDGE queues; NOT gpsimd) ----
        wb_sb = cpool.tile([P, _WB_LEN], BF16)
        fb_sb = cpool.tile([P, T + 2 * NCORES], F32)
        csn_sb = cpool.tile([P, NCH, 2, T], BF16)
        tt_sb = cpool.tile([P, NCH, 2, T], BF16)
        nc.sync.dma_start(wb_sb[:], wb_d)
        nc.sync.dma_start(fb_sb[:], fb_d)
        for c in range(NCH):
            nc.scalar.dma_start(csn_sb[:, c], csn_d[:, c])
        for c in range(NCH):
            nc.scalar.dma_start(tt_sb[:, c], tt_d[:, c])
        rrep = fb_sb[:, 0:T]
        wgr = fb_sb[:, T:T + NCORES]
        wgi = fb_sb[:, T + NCORES:T + 2 * NCORES]
        crt = wb_sb[:, _WB_CRT:_WB_CRT + H]
        cit = wb_sb[:, _WB_CIT:_WB_CIT + H]

        # ---- span-persistent state ----
        yr_sb = spool.tile([P, LC], BF16)         # scan outputs (rotated basis)
        yi_sb = spool.tile([P, LC], BF16)

        # =============== phase 1: Bu, rotate, scan ===============
        for c in range(NCH):
            t0 = c * T
            utc = wpool.tile([P, 2, T], BF16, tag="utc")
            nc.sync.dma_start(utc[:], ut_d[:, :, t0:t0 + T])
            pbr = ppool.tile([P, T], F32, tag="pbr")
            pbi = ppool.tile([P, T], F32, tag="pbi")
            nc.tensor.matmul(pbr[:], wb_sb[:, _WB_BTR0:_WB_BTR0 + P],
                             utc[:, 0, :], start=True, stop=False)
            nc.tensor.matmul(pbr[:], wb_sb[:, _WB_BTR1:_WB_BTR1 + P],
                             utc[:, 1, :], start=False, stop=True)
            nc.tensor.matmul(pbi[:], wb_sb[:, _WB_BTI0:_WB_BTI0 + P],
                             utc[:, 0, :], start=True, stop=False)
            nc.tensor.matmul(pbi[:], wb_sb[:, _WB_BTI1:_WB_BTI1 + P],
                             utc[:, 1, :], start=False, stop=True)
            cs = csn_sb[:, c, 0, :]
            sn = csn_sb[:, c, 1, :]
            # PSUM -> SBUF bf16 on Act (gpsimd cannot touch PSUM; bf16
            # doubles DVE rate downstream)
            bur = wpool.tile([P, T], BF16, tag="bur")
            bui = wpool.tile([P, T], BF16, tag="bui")
            nc.scalar.copy(bur[:], pbr[:])
            nc.scalar.copy(bui[:], pbi[:])
            # w = e^{-i th t} * Bu
            m1 = wpool.tile([P, T], BF16, tag="m1")
            m2 = wpool.tile([P, T], BF16, tag="m2")
            m3 = wpool.tile([P, T], BF16, tag="m3")
            m4 = wpool.tile([P, T], BF16, tag="m4")
            wr = wpool.tile([P, T], BF16, tag="wr")
            wi = wpool.tile([P, T], BF16, tag="wi")
            nc.gpsimd.tensor_mul(m4[:], sn, bur[:])
            nc.vector.tensor_mul(m1[:], cs, bur[:])
            nc.vector.tensor_mul(m2[:], sn, bui[:])
            nc.vector.tensor_add(wr[:], m1[:], m2[:])
            nc.vector.tensor_mul(m3[:], cs, bui[:])
            nc.gpsimd.tensor_sub(wi[:], m3[:], m4[:])
            # chained scans (span-global basis: init = prev chunk's last col)
            init_r = 0.0 if c == 0 else yr_sb[:, t0 - 1:t0]
            init_i = 0.0 if c == 0 else yi_sb[:, t0 - 1:t0]
            nc.vector.tensor_tensor_scan(
                yr_sb[:, t0:t0 + T], rrep, wr[:], init_r, op0=MUL, op1=ADD)
            nc.vector.tensor_tensor_scan(
                yi_sb[:, t0:t0 + T], rrep, wi[:], init_i, op0=MUL, op1=ADD)

        # =============== cross-core carry exchange ===============
        e_loc = dpool.tile([P, 2], F32)
        e_all = dpool.tile([NCORES * P, 2], F32)
        e_sb = cpool.tile([P, 2], F32)
        nc.vector.tensor_copy(e_sb[:, 0:1], yr_sb[:, LC - 1:LC])
        nc.vector.tensor_copy(e_sb[:, 1:2], yi_sb[:, LC - 1:LC])
        nc.gpsimd.dma_start(e_loc[:], e_sb[:])
        if single:
            nc.gpsimd.dma_start(e_all[0:P, :], e_loc[:])
        else:
            nc.gpsimd.collective_compute(
                "AllGather", mybir.AluOpType.bypass,
                replica_groups=[list(range(NCORES))],
                ins=[e_loc.opt()], outs=[e_all.opt()])
        eall_sb = cpool.tile([P, 2 * NCORES], F32)
        for j in range(NCORES):
            nc.gpsimd.dma_start(eall_sb[:, 2 * j:2 * j + 2],
                                e_all[j * P:(j + 1) * P, :])
        # Cin = sum_j W'_j * E_j   (complex, W' host-precomputed per core)
        er_v = eall_sb[:, 0:2 * NCORES:2]
        ei_v = eall_sb[:, 1:2 * NCORES:2]
        ta = cpool.tile([P, NCORES], F32)
        tb = cpool.tile([P, NCORES], F32)
        cr = cpool.tile([P, 1], F32)
        ci = cpool.tile([P, 1], F32)
        nc.vector.tensor_mul(ta[:], wgr, er_v)
        nc.vector.tensor_mul(tb[:], wgi, ei_v)
        nc.vector.tensor_sub(ta[:], ta[:], tb[:])
        nc.vector.tensor_reduce(cr[:], ta[:], axis=AX, op=ADD)
        nc.vector.tensor_mul(ta[:], wgi, er_v)
        nc.vector.tensor_mul(tb[:], wgr, ei_v)
        nc.vector.tensor_add(ta[:], ta[:], tb[:])
        nc.vector.tensor_reduce(ci[:], ta[:], axis=AX, op=ADD)
        # fold Cin into output weights: W1 = crt*cr + cit*ci, W2 = cit*cr - crt*ci
        w1 = cpool.tile([P, H], BF16)
        w2 = cpool.tile([P, H], BF16)
        tw1 = cpool.tile([P, H], BF16)
        tw2 = cpool.tile([P, H], BF16)
        nc.vector.tensor_scalar_mul(tw1[:], crt, cr[:])
        nc.vector.tensor_scalar_mul(tw2[:], cit, ci[:])
        nc.vector.tensor_add(w1[:], tw1[:], tw2[:])
        nc.vector.tensor_scalar_mul(tw1[:], cit, cr[:])
        nc.vector.tensor_scalar_mul(tw2[:], crt, ci[:])
        nc.vector.tensor_sub(w2[:], tw1[:], tw2[:])

        # =============== phase 2: unrotate, project (+carry via W1/W2) =====
        for c in range(NCH):
            t0 = c * T
            cs = csn_sb[:, c, 0, :]
            sn = csn_sb[:, c, 1, :]
            # x = e^{+i th t} * y
            n1 = wpool.tile([P, T], BF16, tag="n1")
            n2 = wpool.tile([P, T], BF16, tag="n2")
            n3 = wpool.tile([P, T], BF16, tag="n3")
            n4 = wpool.tile([P, T], BF16, tag="n4")
            xr = wpool.tile([P, T], BF16, tag="xr")
            xi = wpool.tile([P, T], BF16, tag="xi")
            nc.gpsimd.tensor_mul(n2[:], sn, yi_sb[:, t0:t0 + T])
            nc.vector.tensor_mul(n1[:], cs, yr_sb[:, t0:t0 + T])
            nc.vector.tensor_sub(xr[:], n1[:], n2[:])
            nc.gpsimd.tensor_mul(n4[:], sn, yr_sb[:, t0:t0 + T])
            nc.vector.tensor_mul(n3[:], cs, yi_sb[:, t0:t0 + T])
            nc.vector.tensor_add(xi[:], n3[:], n4[:])
            # ys[t,h] = 2Re(C x)[t,h] + carry correction (TR W1 + TI W2)
            po = opool.tile([P, 4, H], F32, tag="po")
            ob = wpool.tile([P, 4, H], BF16, tag="ob")
            for a in range(4):
                sl = slice(a * P, (a + 1) * P)
                nc.tensor.matmul(po[:, a, :], xr[:, sl], crt,
                                 start=True, stop=False)
                nc.tensor.matmul(po[:, a, :], xi[:, sl], cit,
                                 start=False, stop=False)
                nc.tensor.matmul(po[:, a, :], tt_sb[:, c, 0, sl], w1[:],
                                 start=False, stop=False)
                nc.tensor.matmul(po[:, a, :], tt_sb[:, c, 1, sl], w2[:],
                                 start=False, stop=True)
            nc.scalar.copy(ob[:], po[:])
            nc.sync.dma_start(out_d[c], ob[:])


_NC_CACHE = {}


def _get_nc():
    if "nc" not in _NC_CACHE:
        _NC_CACHE["nc"] = _build_kernel()
    return _NC_CACHE["nc"]


def _host_prep(Lambda_re, Lambda_im, B, C, D, log_step, input_sequence):
    """f64 host-side parameter/table preparation -> per-core input maps."""
    Lam = Lambda_re.astype(np.float64) + 1j * Lambda_im.astype(np.float64)
    step = np.exp(log_step.astype(np.float64))
    A = np.exp(Lam * step)                        # (P,)
    r = np.abs(A)
    th = np.imag(Lam * step)
    Bt = B[..., 0].astype(np.float64) + 1j * B[..., 1].astype(np.float64)
    Bbar = ((A - 1.0) / Lam)[:, None] * Bt        # (P, H)
    Ct = C[..., 0].astype(np.float64) + 1j * C[..., 1].astype(np.float64)  # (H, P)

    s = np.arange(LC, dtype=np.float64)
    ang = th[:, None] * s[None, :]
    cs = np.cos(ang)
    sn = np.sin(ang)
    csn = np.stack([cs.reshape(P, NCH, T), sn.reshape(P, NCH, T)],
                   axis=2).astype(NPBF16)          # [P, NCH, 2, T]
    # TR/TI = Re/Im(A^{t+1}) = r^{t+1} (cos, sin)(th (t+1))
    rp = np.exp(np.log(r)[:, None] * (s[None, :] + 1.0))
    ang1 = th[:, None] * (s[None, :] + 1.0)
    tr = rp * np.cos(ang1)
    ti = rp * np.sin(ang1)
    tt = np.stack([tr.reshape(P, NCH, T), ti.reshape(P, NCH, T)],
                  axis=2).astype(NPBF16)           # [P, NCH, 2, T]

    Br = np.real(Bbar).T                          # (256h, 128p)
    Bi = np.imag(Bbar).T
    crt = 2.0 * np.real(Ct).T                     # (128p, 256h)
    cit = -2.0 * np.imag(Ct).T
    wb = np.concatenate([Br[0:P], Br[P:H], Bi[0:P], Bi[P:H],
                         crt, cit], axis=1).astype(NPBF16)

    rrep = np.broadcast_to(r[:, None], (P, T)).astype(np.float32)
    ALC = A ** LC
    eE = np.exp(1j * th * LC)      # includes the carry-in e^{i th} rotation
    # W'[i, j] = ALC^{i-1-j} * eE  for j < i
    wgc = np.zeros((NCORES, P, NCORES), np.complex128)
    pw = np.ones((P,), np.complex128)
    for k in range(NCORES - 1):
        w = pw * eE
        for j in range(NCORES - 1 - k):
            wgc[j + k + 1, :, j] = w
        pw = pw * ALC

    ub = input_sequence.astype(NPBF16)
    uT = ub.T                                     # (256, L) view

    in_maps = []
    for i in range(NCORES):
        utc = np.ascontiguousarray(
            uT[:, i * LC:(i + 1) * LC].reshape(2, P, LC).transpose(1, 0, 2))
        fb = np.concatenate(
            [rrep,
             np.ascontiguousarray(np.real(wgc[i])).astype(np.float32),
             np.ascontiguousarray(np.imag(wgc[i])).astype(np.float32)],
            axis=1)
        in_maps.append({
            "ut": utc,
            "csn": csn,
            "tt": tt,
            "wb": wb,
            "fb": fb,
        })
    return in_maps


def kernel(Lambda_re, Lambda_im, B, C, D, log_step, input_sequence):
    in_maps = _host_prep(Lambda_re, Lambda_im, B, C, D, log_step,
                         input_sequence)
    nc = _get_nc()
    res = run_bass_kernel_spmd(nc, in_maps, list(range(NCORES)))
    out = np.concatenate(
        [_unscramble(res.results[i]["out"]) for i in range(NCORES)], axis=0)
    # D*u is cheaper on the host than 8 PE matmuls per chunk on device
    out += D.astype(np.float32) * input_sequence
    return out


def _unscramble(out_arr):
    """device layout [NCH, P, 4, H] (p-major) bf16 -> time-major [LC, H] f32"""
    return (np.asarray(out_arr).astype(np.float32)
            .transpose(0, 2, 1, 3).reshape(LC, H))


if __name__ == "__main__":
    pass


# revision 14
# speedup vs baseline: 1.2155x; 1.1900x over previous
"""Trainium2 Bass kernel for a continuous-time diagonal SSM layer (S5/S4D-style).

Math (see reference):
    A = exp(Lambda * step)                 (P,) complex, |A| = r, arg = theta
    Bu[t] = B_bar @ u[t]                   (L, P) complex
    x[t]  = A * x[t-1] + Bu[t]             diagonal complex scan
    ys    = 2 Re(C_tilde @ x) + D * u

Kernel strategy (8 cores, sequence-parallel over L):
  * core i owns a contiguous span of LC = L/8 timesteps, chunked by T=512.
  * rotation trick with a SPAN-GLOBAL basis: with A = r e^{i.th}, define
    y[t] = e^{-i.th.t} x[t] (t = span-local index).  y obeys a REAL
    per-partition recurrence y[t] = r y[t-1] + w[t], w[t] = e^{-i.th.t} Bu[t],
    mapping onto the DVE tensor_tensor_scan.  Chunk scans chain by passing
    initial = previous chunk's last output column.
  * cross-core carry: each core AllGathers its span-final scan state (1KB),
    combines with host-precomputed decay weights into a per-partition complex
    scalar Cin = cr + i*ci.  The correction x[t] += A^{t+1} Cin is applied IN
    OUTPUT SPACE: ys_corr[t,h] = sum_p TR[p,t] W1[p,h] + TI[p,t] W2[p,h]
    with TR/TI = Re/Im A^{t+1} (host tables) and W1 = crt*cr + cit*ci,
    W2 = cit*cr - crt*ci folded once after the collective - two extra PE
    matmuls per output subtile, zero extra DVE passes.
  * D*u is added on the HOST after gather (saves 8 PE matmuls per chunk).
  * engine balance (cost-model driven): DVE 297ns/tensor-tensor (bf16 2x),
    594ns/scan; Pool 1111ns/op (no PSUM access allowed!); Act copies
    754-1506ns.  Per chunk: DVE gets 3 mults + 1 add + 2 scans (p1), 2 mults
    + 2 add/sub (p2); Pool gets 2 ops per phase; Act does PSUM->SBUF copies;
    PE all matmuls; const/in/out DMAs ride SP + Act HWDGE queues (issuing
    big DMAs from gpsimd stalls the Pool ENGINE for the whole transfer).
"""

import numpy as np
import ml_dtypes

import concourse.bass as bass
import concourse.bacc as bacc
import concourse.tile as tile
import concourse.mybir as mybir
from concourse.bass_utils import run_bass_kernel_spmd

F32 = mybir.dt.float32
BF16 = mybir.dt.bfloat16
NPBF16 = ml_dtypes.bfloat16
AX = mybir.AxisListType.X
MUL = mybir.AluOpType.mult
ADD = mybir.AluOpType.add

L, H, P = 32768, 256, 128
NCORES = 8
LC = L // NCORES          # 4096 timesteps per core
T = 512                   # chunk length
NCH = LC // T             # 8 chunks per core

# weight-blob layout (free-dim offsets, all bf16, partition=128)
_WB_BTR0 = 0          # Re(Bbar).T rows   0:128  -> [128h, 128p]
_WB_BTR1 = 128        # Re(Bbar).T rows 128:256
_WB_BTI0 = 256        # Im(Bbar).T rows   0:128
_WB_BTI1 = 384        # Im(Bbar).T rows 128:256
_WB_CRT = 512         # 2*Re(C).T   [128p, 256h]
_WB_CIT = 768         # -2*Im(C).T  [128p, 256h]
_WB_LEN = 1024


def _build_kernel(single=False):
    nc = bacc.Bacc(
        "TRN2", target_bir_lowering=False, debug=False,
        enable_asserts=False, num_devices=1 if single else NCORES,
    )
    # ---------------- per-core external I/O ----------------
    ut_d = nc.dram_tensor("ut", [P, 2, LC], BF16, kind="ExternalInput").ap()
    csn_d = nc.dram_tensor("csn", [P, NCH, 2, T], BF16, kind="ExternalInput").ap()
    tt_d = nc.dram_tensor("tt", [P, NCH, 2, T], BF16, kind="ExternalInput").ap()
    wb_d = nc.dram_tensor("wb", [P, _WB_LEN], BF16, kind="ExternalInput").ap()
    fb_d = nc.dram_tensor("fb", [P, T + 2 * NCORES], F32, kind="ExternalInput").ap()
    # natural (p, a, h) order to match the SBUF tile; host re-lays out
    out_d = nc.dram_tensor("out", [NCH, P, 4, H], BF16, kind="ExternalOutput").ap()

    with tile.TileContext(nc) as tc:
        _body(tc, nc, ut_d, csn_d, tt_d, wb_d, fb_d, out_d, single=single)
    nc.compile()
    return nc


def _body(tc, nc, ut_d, csn_d, tt_d, wb_d, fb_d, out_d, single=False):
    with (
        tc.tile_pool(name="const", bufs=1) as cpool,
        tc.tile_pool(name="span", bufs=1) as spool,
        tc.tile_pool(name="work", bufs=3) as wpool,
        tc.tile_pool(name="pbu", bufs=2, space="PSUM") as ppool,
        tc.tile_pool(name="pout", bufs=2, space="PSUM") as opool,
        tc.tile_pool(name="dram", bufs=1, space="DRAM") as dpool,
    ):
        # ---- constants (SP + Act HWDGE queues; NOT gpsimd).  Order matters:
        # wb + chunk-0 inputs first so PE starts ~1.5us in; tt (phase-2-only)
        # trickles in during phase 1.
        wb_sb = cpool.tile([P, _WB_LEN], BF16)
        fb_sb = cpool.tile([P, T + 2 * NCORES], F32)
        csn_sb = cpool.tile([P, NCH, 2, T], BF16)
        tt_sb = cpool.tile([P, NCH, 2, T], BF16)
        nc.sync.dma_start(wb_sb[:], wb_d)
        nc.scalar.dma_start(fb_sb[:], fb_d)
        rrep = fb_sb[:, 0:T]
        wgr = fb_sb[:, T:T + NCORES]
        wgi = fb_sb[:, T + NCORES:T + 2 * NCORES]
        crt = wb_sb[:, _WB_CRT:_WB_CRT + H]
        cit = wb_sb[:, _WB_CIT:_WB_CIT + H]

        # ---- span-persistent state ----
        yr_sb = spool.tile([P, LC], BF16)         # scan outputs (rotated basis)
        yi_sb = spool.tile([P, LC], BF16)
        xr_sb = spool.tile([P, LC], BF16)         # unrotated state
        xi_sb = spool.tile([P, LC], BF16)

        # =============== phase 1: Bu, rotate, scan ===============
        for c in range(NCH):
            t0 = c * T
            utc = wpool.tile([P, 2, T], BF16, tag="utc")
            nc.sync.dma_start(utc[:], ut_d[:, :, t0:t0 + T])
            nc.scalar.dma_start(csn_sb[:, c], csn_d[:, c])
            # tt is phase-2-only: stream it in behind the phase-1 deps
            nc.scalar.dma_start(tt_sb[:, c], tt_d[:, c])
            pbr = ppool.tile([P, T], F32, tag="pbr")
            pbi = ppool.tile([P, T], F32, tag="pbi")
            nc.tensor.matmul(pbr[:], wb_sb[:, _WB_BTR0:_WB_BTR0 + P],
                             utc[:, 0, :], start=True, stop=False)
            nc.tensor.matmul(pbr[:], wb_sb[:, _WB_BTR1:_WB_BTR1 + P],
                             utc[:, 1, :], start=False, stop=True)
            nc.tensor.matmul(pbi[:], wb_sb[:, _WB_BTI0:_WB_BTI0 + P],
                             utc[:, 0, :], start=True, stop=False)
            nc.tensor.matmul(pbi[:], wb_sb[:, _WB_BTI1:_WB_BTI1 + P],
                             utc[:, 1, :], start=False, stop=True)
            cs = csn_sb[:, c, 0, :]
            sn = csn_sb[:, c, 1, :]
            # PSUM -> SBUF bf16 on Act (gpsimd cannot touch PSUM; bf16
            # doubles DVE rate downstream)
            bur = wpool.tile([P, T], BF16, tag="bur")
            bui = wpool.tile([P, T], BF16, tag="bui")
            nc.scalar.copy(bur[:], pbr[:])
            nc.scalar.copy(bui[:], pbi[:])
            # w = e^{-i th t} * Bu
            m1 = wpool.tile([P, T], BF16, tag="m1")
            m2 = wpool.tile([P, T], BF16, tag="m2")
            m3 = wpool.tile([P, T], BF16, tag="m3")
            m4 = wpool.tile([P, T], BF16, tag="m4")
            wr = wpool.tile([P, T], BF16, tag="wr")
            wi = wpool.tile([P, T], BF16, tag="wi")
            nc.gpsimd.tensor_mul(m4[:], sn, bur[:])
            nc.vector.tensor_mul(m1[:], cs, bur[:])
            nc.vector.tensor_mul(m2[:], sn, bui[:])
            nc.vector.tensor_add(wr[:], m1[:], m2[:])
            nc.vector.tensor_mul(m3[:], cs, bui[:])
            nc.gpsimd.tensor_sub(wi[:], m3[:], m4[:])
            # chained scans (span-global basis: init = prev chunk's last col)
            init_r = 0.0 if c == 0 else yr_sb[:, t0 - 1:t0]
            init_i = 0.0 if c == 0 else yi_sb[:, t0 - 1:t0]
            nc.vector.tensor_tensor_scan(
                yr_sb[:, t0:t0 + T], rrep, wr[:], init_r, op0=MUL, op1=ADD)
            nc.vector.tensor_tensor_scan(
                yi_sb[:, t0:t0 + T], rrep, wi[:], init_i, op0=MUL, op1=ADD)

        # =============== kick off cross-core carry exchange ===============
        # (the AllGather rides the gpsimd queue while DVE/Pool unrotate)
        e_loc = dpool.tile([P, 2], F32)
        e_all = dpool.tile([NCORES, P, 2], F32)
        e_sb = cpool.tile([P, 2], F32)
        nc.vector.tensor_copy(e_sb[:, 0:1], yr_sb[:, LC - 1:LC])
        nc.vector.tensor_copy(e_sb[:, 1:2], yi_sb[:, LC - 1:LC])
        nc.gpsimd.dma_start(e_loc[:], e_sb[:])
        if single:
            nc.gpsimd.dma_start(e_all[0], e_loc[:])
        else:
            nc.gpsimd.collective_compute(
                "AllGather", mybir.AluOpType.bypass,
                replica_groups=[list(range(NCORES))],
                ins=[e_loc.opt()], outs=[e_all.opt()])

        # =============== phase 2a: unrotate (collective-independent) =======
        for c in range(NCH):
            t0 = c * T
            cs = csn_sb[:, c, 0, :]
            sn = csn_sb[:, c, 1, :]
            n1 = wpool.tile([P, T], BF16, tag="n1")
            n2 = wpool.tile([P, T], BF16, tag="n2")
            n3 = wpool.tile([P, T], BF16, tag="n3")
            n4 = wpool.tile([P, T], BF16, tag="n4")
            nc.gpsimd.tensor_mul(n2[:], sn, yi_sb[:, t0:t0 + T])
            nc.vector.tensor_mul(n1[:], cs, yr_sb[:, t0:t0 + T])
            nc.vector.tensor_sub(xr_sb[:, t0:t0 + T], n1[:], n2[:])
            nc.vector.tensor_mul(n3[:], cs, yi_sb[:, t0:t0 + T])
            nc.vector.tensor_mul(n4[:], sn, yr_sb[:, t0:t0 + T])
            nc.vector.tensor_add(xi_sb[:, t0:t0 + T], n3[:], n4[:])

        # =============== carry combine -> W1/W2 ===============
        eall_sb = cpool.tile([P, NCORES, 2], F32)
        nc.sync.dma_start(eall_sb[:],
                          e_all[:, :, :].rearrange("j p c -> p j c"))
        er_v = eall_sb[:, :, 0]
        ei_v = eall_sb[:, :, 1]
        ta = cpool.tile([P, NCORES], F32)
        tb = cpool.tile([P, NCORES], F32)
        cr = cpool.tile([P, 1], F32)
        ci = cpool.tile([P, 1], F32)
        nc.vector.tensor_mul(ta[:], wgr, er_v)
        nc.vector.tensor_mul(tb[:], wgi, ei_v)
        nc.vector.tensor_sub(ta[:], ta[:], tb[:])
        nc.vector.tensor_reduce(cr[:], ta[:], axis=AX, op=ADD)
        nc.vector.tensor_mul(ta[:], wgi, er_v)
        nc.vector.tensor_mul(tb[:], wgr, ei_v)
        nc.vector.tensor_add(ta[:], ta[:], tb[:])
        nc.vector.tensor_reduce(ci[:], ta[:], axis=AX, op=ADD)
        # fold Cin into output weights: W1 = crt*cr + cit*ci, W2 = cit*cr - crt*ci
        w1 = cpool.tile([P, H], BF16)
        w2 = cpool.tile([P, H], BF16)
        tw1 = cpool.tile([P, H], BF16)
        tw2 = cpool.tile([P, H], BF16)
        nc.vector.tensor_scalar_mul(tw1[:], crt, cr[:])
        nc.vector.tensor_scalar_mul(tw2[:], cit, ci[:])
        nc.vector.tensor_add(w1[:], tw1[:], tw2[:])
        nc.vector.tensor_scalar_mul(tw1[:], cit, cr[:])
        nc.vector.tensor_scalar_mul(tw2[:], crt, ci[:])
        nc.vector.tensor_sub(w2[:], tw1[:], tw2[:])

        # =============== phase 2b: project (+carry via W1/W2) ===============
        for c in range(NCH):
            t0 = c * T
            po = opool.tile([P, 4, H], F32, tag="po")
            ob = wpool.tile([P, 4, H], BF16, tag="ob")
            for a in range(4):
                sl = slice(t0 + a * P, t0 + (a + 1) * P)
                nc.tensor.matmul(po[:, a, :], xr_sb[:, sl], crt,
                                 start=True, stop=False)
                nc.tensor.matmul(po[:, a, :], xi_sb[:, sl], cit,
                                 start=False, stop=False)
                nc.tensor.matmul(po[:, a, :], tt_sb[:, c, 0, a * P:(a + 1) * P],
                                 w1[:], start=False, stop=False)
                nc.tensor.matmul(po[:, a, :], tt_sb[:, c, 1, a * P:(a + 1) * P],
                                 w2[:], start=False, stop=True)
            nc.scalar.copy(ob[:], po[:])
            nc.sync.dma_start(out_d[c], ob[:])


_NC_CACHE = {}


def _get_nc():
    if "nc" not in _NC_CACHE:
        _NC_CACHE["nc"] = _build_kernel()
    return _NC_CACHE["nc"]


def _host_prep(Lambda_re, Lambda_im, B, C, D, log_step, input_sequence):
    """f64 host-side parameter/table preparation -> per-core input maps."""
    Lam = Lambda_re.astype(np.float64) + 1j * Lambda_im.astype(np.float64)
    step = np.exp(log_step.astype(np.float64))
    A = np.exp(Lam * step)                        # (P,)
    r = np.abs(A)
    th = np.imag(Lam * step)
    Bt = B[..., 0].astype(np.float64) + 1j * B[..., 1].astype(np.float64)
    Bbar = ((A - 1.0) / Lam)[:, None] * Bt        # (P, H)
    Ct = C[..., 0].astype(np.float64) + 1j * C[..., 1].astype(np.float64)  # (H, P)

    s = np.arange(LC, dtype=np.float64)
    ang = th[:, None] * s[None, :]
    cs = np.cos(ang)
    sn = np.sin(ang)
    csn = np.stack([cs.reshape(P, NCH, T), sn.reshape(P, NCH, T)],
                   axis=2).astype(NPBF16)          # [P, NCH, 2, T]
    # TR/TI = Re/Im(A^{t+1}) = r^{t+1} (cos, sin)(th (t+1))
    rp = np.exp(np.log(r)[:, None] * (s[None, :] + 1.0))
    ang1 = th[:, None] * (s[None, :] + 1.0)
    tr = rp * np.cos(ang1)
    ti = rp * np.sin(ang1)
    tt = np.stack([tr.reshape(P, NCH, T), ti.reshape(P, NCH, T)],
                  axis=2).astype(NPBF16)           # [P, NCH, 2, T]

    Br = np.real(Bbar).T                          # (256h, 128p)
    Bi = np.imag(Bbar).T
    crt = 2.0 * np.real(Ct).T                     # (128p, 256h)
    cit = -2.0 * np.imag(Ct).T
    wb = np.concatenate([Br[0:P], Br[P:H], Bi[0:P], Bi[P:H],
                         crt, cit], axis=1).astype(NPBF16)

    rrep = np.broadcast_to(r[:, None], (P, T)).astype(np.float32)
    ALC = A ** LC
    eE = np.exp(1j * th * LC)      # includes the carry-in e^{i th} rotation
    # W'[i, j] = ALC^{i-1-j} * eE  for j < i
    wgc = np.zeros((NCORES, P, NCORES), np.complex128)
    pw = np.ones((P,), np.complex128)
    for k in range(NCORES - 1):
        w = pw * eE
        for j in range(NCORES - 1 - k):
            wgc[j + k + 1, :, j] = w
        pw = pw * ALC

    ub = input_sequence.astype(NPBF16)
    uT = ub.T                                     # (256, L) view

    in_maps = []
    for i in range(NCORES):
        utc = np.ascontiguousarray(
            uT[:, i * LC:(i + 1) * LC].reshape(2, P, LC).transpose(1, 0, 2))
        fb = np.concatenate(
            [rrep,
             np.ascontiguousarray(np.real(wgc[i])).astype(np.float32),
             np.ascontiguousarray(np.imag(wgc[i])).astype(np.float32)],
            axis=1)
        in_maps.append({
            "ut": utc,
            "csn": csn,
            "tt": tt,
            "wb": wb,
            "fb": fb,
        })
    return in_maps


def kernel(Lambda_re, Lambda_im, B, C, D, log_step, input_sequence):
    in_maps = _host_prep(Lambda_re, Lambda_im, B, C, D, log_step,
                         input_sequence)
    nc = _get_nc()
    res = run_bass_kernel_spmd(nc, in_maps, list(range(NCORES)))
    out = np.concatenate(
        [_unscramble(res.results[i]["out"]) for i in range(NCORES)], axis=0)
    # D*u is cheaper on the host than 8 PE matmuls per chunk on device
    out += D.astype(np.float32) * input_sequence
    return out


def _unscramble(out_arr):
    """device layout [NCH, P, 4, H] (p-major) bf16 -> time-major [LC, H] f32"""
    return (np.asarray(out_arr).astype(np.float32)
            .transpose(0, 2, 1, 3).reshape(LC, H))


if __name__ == "__main__":
    pass
